# revision 53
# baseline (speedup 1.0000x reference)
"""AdaptedAttention (llama + adaption-prompt) on 8 TRN2 NeuronCores.

Sharding: tensor-parallel over heads (2 heads/core), zero device collectives
(measured AllGather ~92us/rank through this stack, so the cross-core
reduction of the 8 per-core output partials happens on the host).

Per core (all on-device tensors fp16 except PSUM/f32 glue; fp16 beats bf16
on precision at identical PE/DVE throughput, buying error budget):
  - qT/kT/V projections for its 2 heads from a pre-transposed X [d, s]
  - RoPE in [hd, s] layout from HOST-built cos/sin fp16 tables. On-device
    tables would need ACT Sin, and Sin/Exp never share an activation table
    (1283ns reload per switch, every chunk). The rotate-half sign is folded
    into the host sin table; the cross-partition-base products read the
    projection result straight from PSUM (the equal-base-partition verifier
    rule only applies to SBUF+SBUF operand pairs).
  - attention as S^T = K @ Q^T tiles ([k part, q free]); exp on ACT; causal
    handled by skipping k-tiles above the diagonal, streaming only the
    valid q-columns of diagonal tiles (width QC-128j), and a 0/1 mask on
    each diagonal tile's own 128-col triangle
  - NO PE row-sum/broadcast matmuls for softmax denominators: exp tiles
    accumulate into even/odd fp16 tiles (h0 on the idle GPSIMD, h1 on DVE,
    split so neither engine's add rate binds the k-loop), merged by one DVE
    add; one [128,1]-ones matmul -> denominator row; reciprocal_approx_fast
    + gpsimd partition_broadcast replace ones-outer-product broadcasts
  - adapter path (L=10): adaption gate folded into adapter-V at load time
  - output projection -> full-shape partial [d, s] fp16 (half the f32
    store traffic), staged PSUM->SBUF in dt-pairs alternating ACT/DVE,
    written on the SP HWDGE queue (inputs share it; ACT's queue carries
    only small weight/table loads so its sequencer stays on exp)

Scheduling (the core of the speedup): the exp's ~612ns/tile ACT cost vs
852ns of PE work per k-iter pair means a naive k-loop is ACT-round-trip
bound, and every PE gap also resets the tensor engine's p-state ramp
(full 2.4GHz only after 3us of continuous execution). So:
  - both heads' k-loops are interleaved at iteration level (doubles PE work
    per st-slot round trip; the 2-slot PSUM ping-pong never binds)
  - the NEXT chunk's projection matmuls and the PREVIOUS chunk's output-
    projection matmuls are kept in a queue of single-instruction units and
    woven into the k-loop / combine sections one per iteration, keeping the
    PE stream dense end to end (emission-order = dependency-order, so
    deferred producer units are force-emitted just before the diagonal
    section that consumes them)
  - the softmax combine is stage-interleaved across heads so its serial
    DVE chain (recips, t1/t2) overlaps Pool broadcasts and woven PE work
PSUM = exactly 8 banks via 4 rotating tags (qk/st/acc/vo x2).

TimelineSim (HW-calibrated cost model): 185us single-shot, 151us/rep
steady-state (PE ~99% occupied at the fp16 column floor) vs 362us for the
previous kernel. Host-measured wall marginals through the axon tunnel are
noise-dominated (dispatch variance >> kernel time) and not trustworthy.
"""

import math
from collections import deque
import numpy as np

import concourse.bass as bass
import concourse.bacc as bacc
import concourse.mybir as mybir
import concourse.tile as tile
from concourse.bass_utils import run_bass_kernel_spmd

F16 = mybir.dt.float16
BF16 = mybir.dt.bfloat16
F32 = mybir.dt.float32
NP_F16 = np.float16
NP_BF16 = mybir.dt.np(BF16)


class Cfg:
    def __init__(self, s=2048, d=2048, L=10, n_cores=8, n_heads=16, rope_base=10000.0):
        self.s, self.d, self.L = s, d, L
        self.n_cores = n_cores
        self.n_heads = n_heads
        self.rope_base = rope_base
        self.hd = 128                      # head dim (fixed)
        self.hpc = n_heads // n_cores      # heads per core
        self.dh = self.hpc * self.hd       # local head-dim cols per core
        self.nd = d // 128                 # contraction chunks
        self.QC = 512                      # q-chunk width
        self.ns = s // self.QC             # q-chunks
        self.nst = s // 128                # s tiles (k tiles)
        self.kpq = self.QC // 128          # k-tiles per q-chunk
        assert self.hpc * n_cores == n_heads and d % 128 == 0 and s % self.QC == 0


WEAVE_RATE = 1         # proj matmuls woven per attention k-iter
ACC_ON_GPSIMD = True  # est-sum accumulation engine for non-diag tiles


def build(cfg: Cfg, nrep: int = 1):
    c = cfg
    nc = bacc.Bacc(None, target_bir_lowering=False, num_devices=c.n_cores)

    # ---------------- external I/O (per-core shards) ----------------
    xt_d = nc.dram_tensor("xt", [c.d, c.s], F16, kind="ExternalInput")
    wqt_d = nc.dram_tensor("wqt", [c.d, c.dh], F16, kind="ExternalInput")
    wkt_d = nc.dram_tensor("wkt", [c.d, c.dh], F16, kind="ExternalInput")
    wvt_d = nc.dram_tensor("wvt", [c.d, c.dh], F16, kind="ExternalInput")
    wot_d = nc.dram_tensor("wot", [c.dh, c.d], F16, kind="ExternalInput")
    apt_d = nc.dram_tensor("apt", [c.d, c.L], F16, kind="ExternalInput")
    gatec_d = nc.dram_tensor("gatec", [128, 1], F32, kind="ExternalInput")
    cosd_d = nc.dram_tensor("cosd", [c.hd, c.s], F16, kind="ExternalInput")
    sind_d = nc.dram_tensor("sind", [c.hd, c.s], F16, kind="ExternalInput")
    out_d = nc.dram_tensor("out", [c.d, c.s], F16, kind="ExternalOutput")

    # diagonal-tile causal masks: mask[k, j, q] = 1 if k <= q - 128*j
    kk = np.arange(128)[:, None, None]
    jj = np.arange(c.kpq)[None, :, None]
    qq = np.arange(c.QC)[None, None, :]
    masks_np = (kk <= qq - 128 * jj).astype(NP_F16)  # [128, kpq, QC]
    masks_d = nc.inline_tensor(masks_np, name="masks")

    scale_s = 1.0 / math.sqrt(c.hd)        # main attention scale
    scale_a = 1.0 / math.sqrt(c.n_heads)   # adapter scale (faithful to ref)

    EXP = mybir.ActivationFunctionType.Exp
    ADD = mybir.AluOpType.add
    SUB = mybir.AluOpType.subtract
    MUL = mybir.AluOpType.mult

    xt_r = xt_d.rearrange("(t p) s -> p t s", p=128)
    wqt_r = wqt_d.rearrange("(t p) m -> p t m", p=128)
    wkt_r = wkt_d.rearrange("(t p) m -> p t m", p=128)
    wvt_r = wvt_d.rearrange("(t p) m -> p t m", p=128)
    wot_r = wot_d.rearrange("(h p) m -> p h m", p=128)
    apt_r = apt_d.rearrange("(t p) m -> p t m", p=128)
    out_r = out_d.rearrange("(t p) s -> p t s", p=128)

    with tile.TileContext(nc) as tc:
        with (
            tc.tile_pool(name="persist", bufs=1) as pp,
            tc.tile_pool(name="work", bufs=2) as wp,
            tc.tile_pool(name="psum", bufs=2, space="PSUM") as psp,
        ):
            for _rep in range(nrep):
                # ---------- loads ----------
                # weights/tables on the ACT HWDGE queue, xt/wvt/wot on the SP
                # queue; first pieces small so proj(0) starts within ~2us
                wqt = pp.tile([128, c.nd, c.dh], F16, tag="wqt")
                wkt = pp.tile([128, c.nd, c.dh], F16, tag="wkt")
                qtr = max(c.nd // 4, 1)
                xtc = []
                for qc in range(c.ns):
                    xtq = pp.tile([128, c.nd, c.QC], F16, tag=f"xt{qc}",
                                  name=f"xtc{qc}")
                    xtc.append(xtq)
                sl0 = slice(0, c.QC)
                for p in range(0, c.nd, qtr):
                    pe = min(p + qtr, c.nd)
                    nc.scalar.dma_start(wqt[:, p:pe, :], wqt_r[:, p:pe, :])
                    nc.sync.dma_start(xtc[0][:, p:pe, :], xt_r[:, p:pe, sl0])
                    nc.scalar.dma_start(wkt[:, p:pe, :], wkt_r[:, p:pe, :])
                cos_t = pp.tile([c.hd, c.s], F16, tag="cos")
                sin_t = pp.tile([c.hd, c.s], F16, tag="sin")
                nc.scalar.dma_start(cos_t[:], cosd_d[:])
                nc.scalar.dma_start(sin_t[:], sind_d[:])
                wvt = pp.tile([128, c.nd, c.dh], F16, tag="wvt")
                nc.sync.dma_start(wvt[:], wvt_r[:])
                masks = pp.tile([128, c.kpq, c.QC], F16, tag="masks")
                nc.scalar.dma_start(masks[:], masks_d[:])
                gatec = pp.tile([128, 1], F32, tag="gatec")
                nc.scalar.dma_start(gatec[:], gatec_d[:])
                apt = pp.tile([128, c.nd, c.L], F16, tag="apt")
                nc.scalar.dma_start(apt[:], apt_r[:])
                for qc in range(1, c.ns):
                    sl = slice(qc * c.QC, (qc + 1) * c.QC)
                    nc.sync.dma_start(xtc[qc][:], xt_r[:, :, sl])
                wot = pp.tile([128, c.hpc, c.d], F16, tag="wot")
                nc.sync.dma_start(wot[:], wot_r[:])

                ones_f16 = pp.tile([128, 1], F16, tag="ones_f16")
                nc.gpsimd.memset(ones_f16[:], 1.0)
                ones_bf = pp.tile([c.L, 1], BF16, tag="ones_bf")
                nc.gpsimd.memset(ones_bf[:], 1.0)

                # ---------- persistent intermediates ----------
                qrot = [pp.tile([128, c.s], F16, tag=f"qrot{h}", name=f"qrot{h}")
                        for h in range(c.hpc)]
                krot = [pp.tile([128, c.s], F16, tag=f"krot{h}", name=f"krot{h}")
                        for h in range(c.hpc)]
                v_sb = pp.tile([128, c.nst, c.dh], F16, tag="v")
                akt = pp.tile([128, c.hpc, c.L], F16, tag="akt")
                av_sb = pp.tile([c.L, c.dh], BF16, tag="av")

                def rope_emit(dst_tile, sl, ps):
                    # dst = x*cos + rotate_half(x)*sin, with the sign of the
                    # first sin half pre-folded into the host sin table. The
                    # cross-partition-base reads are legal because ps is in
                    # PSUM (the equal-base rule applies to SBUF+SBUF pairs).
                    t1 = wp.tile([128, c.QC], F16, tag="rt", bufs=4, name="r_t1")
                    t2 = wp.tile([128, c.QC], F16, tag="rt", bufs=4, name="r_t2")
                    nc.vector.tensor_tensor(t1[:], ps[:], cos_t[:, sl], MUL)
                    nc.vector.tensor_tensor(t2[0:64], ps[64:128],
                                            sin_t[0:64, sl], MUL)
                    nc.vector.tensor_tensor(t2[64:128], ps[0:64],
                                            sin_t[64:128, sl], MUL)
                    nc.vector.tensor_tensor(dst_tile[:, sl], t1[:], t2[:], ADD)

                def make_qk_units(qc, which=("q", "k")):
                    """Projection matmuls + rope for chunk qc, as single-
                    matmul emission units for weaving into attention. All q
                    units are emitted before all k units so the last chunk's
                    k can be deferred into its own attention k-loop."""
                    units = []
                    sl = slice(qc * c.QC, (qc + 1) * c.QC)
                    state = {}
                    for w in which:
                        wt = wqt if w == "q" else wkt
                        rot = qrot if w == "q" else krot
                        for h in range(c.hpc):
                            hsl = slice(h * 128, (h + 1) * 128)
                            for t in range(c.nd):
                                def mm(h=h, w=w, wt=wt, t=t, hsl=hsl):
                                    if t == 0:
                                        state[(h, w)] = psp.tile(
                                            [128, c.QC], F32, tag="qk",
                                            name=f"p{qc}_{h}{w}")
                                    nc.tensor.matmul(
                                        state[(h, w)][:], wt[:, t, hsl],
                                        xtc[qc][:, t, :],
                                        start=(t == 0), stop=(t == c.nd - 1))
                                units.append(mm)

                            def fin(h=h, w=w, rot=rot):
                                rope_emit(rot[h], sl, state[(h, w)])
                            units.append(fin)
                    return units

                def make_v_units(qc):
                    units = []
                    state = {}
                    for st_i in range(c.kpq):
                        for t in range(c.nd):
                            def vmm(st_i=st_i, t=t):
                                if t == 0:
                                    state[st_i] = psp.tile(
                                        [128, c.dh], F32, tag="vo",
                                        name=f"v{qc}_{st_i}")
                                ssl = slice(st_i * 128, (st_i + 1) * 128)
                                nc.tensor.matmul(
                                    state[st_i][:], xtc[qc][:, t, ssl],
                                    wvt[:, t, :],
                                    start=(t == 0), stop=(t == c.nd - 1))
                            units.append(vmm)

                        def vfin(st_i=st_i):
                            gst = qc * c.kpq + st_i
                            nc.scalar.copy(v_sb[:, gst, :], state[st_i][:])
                        units.append(vfin)
                    return units

                WQ = deque()  # entries: (tag, emit_fn)

                def weave(n):
                    for _ in range(n):
                        if WQ:
                            WQ.popleft()[1]()

                def drain():
                    while WQ:
                        WQ.popleft()[1]()

                def ensure_proj():
                    # emit queued units until no projection units remain;
                    # Tile deps are program-order, so every proj write must
                    # be emitted before attention instructions that read it
                    while any(tag == "proj" for tag, _ in WQ):
                        WQ.popleft()[1]()

                def adapter_kv():
                    for h in range(c.hpc):
                        hsl = slice(h * 128, (h + 1) * 128)
                        a_ps = psp.tile([128, c.L], F32, tag="st",
                                        name=f"akt_ps{h}")
                        for t in range(c.nd):
                            nc.tensor.matmul(a_ps[:], wkt[:, t, hsl],
                                             apt[:, t, :],
                                             start=(t == 0), stop=(t == c.nd - 1))
                        nc.scalar.copy(akt[:, h, 0:c.L], a_ps[:])
                    av_ps = psp.tile([c.L, c.dh], F32, tag="st", name="av_ps")
                    for t in range(c.nd):
                        nc.tensor.matmul(av_ps[:], apt[:, t, :], wvt[:, t, :],
                                         start=(t == 0), stop=(t == c.nd - 1))
                    # fold the adaption gate into adapter-V (per-partition scale)
                    nc.scalar.activation(av_sb[:], av_ps[:],
                                         mybir.ActivationFunctionType.Copy,
                                         scale=gatec[0:c.L, :])

                def attn_chunk(qc, ct_tiles):
                    # both heads' k-loops interleaved at iteration level:
                    # doubles the PE work per st-slot round trip (exp latency
                    # no longer binds) and spreads est accumulation across
                    # Pool (h0) and DVE (h1)
                    sl = slice(qc * c.QC, (qc + 1) * c.QC)
                    nkt = (qc + 1) * c.kpq
                    H = c.hpc
                    ctx_ps = [psp.tile([128, c.QC], F32, tag="acc",
                                       name=f"ctx{qc}_{h}") for h in range(H)]
                    eacc = [[wp.tile([128, c.QC], F16, tag="eacc", bufs=8,
                                     name=f"eacc{qc}_{h}_{par}")
                             for par in range(2)] for h in range(H)]
                    st_ps = [psp.tile([128, c.QC], F32, tag="st",
                                      name=f"st{qc}_{h}_0") for h in range(H)]
                    for h in range(H):
                        nc.tensor.matmul(st_ps[h][:], krot[h][:, 0:128],
                                         qrot[h][:, sl], start=True, stop=True)
                    if qc == 0 and nkt > 1:
                        # chunk 0's kt=1 init write is narrow; zero the odd
                        # accumulators so the skipped columns are defined
                        for h in range(H):
                            nc.gpsimd.memset(eacc[h][1][:], 0.0)

                    def tile_geom(kt):
                        # diagonal tile j: only q-cols >= 128*j are unmasked;
                        # stream just those (plus its own 128-col triangle
                        # still needs the 0/1 mask)
                        j = kt - qc * c.kpq
                        off = max(j, 0) * 128
                        return j, off, c.QC - off

                    for kt in range(nkt):
                        j, off, w = tile_geom(kt)
                        if j == -1 or (nkt == c.kpq and kt == 0):
                            ensure_proj()  # diag k/V writes must be emitted
                            # before their readers below
                        st_cur = [st_ps[h] for h in range(H)]
                        cur_w = w
                        cur_off = off
                        cur_j = j
                        if kt + 1 < nkt:
                            ksl = slice((kt + 1) * 128, (kt + 2) * 128)
                            _, noff, nw = tile_geom(kt + 1)
                            qsl = slice(qc * c.QC + noff, (qc + 1) * c.QC)
                            for h in range(H):
                                st_ps[h] = psp.tile([128, c.QC], F32,
                                                    tag="st",
                                                    name=f"st{qc}_{h}_{kt+1}")
                                nc.tensor.matmul(st_ps[h][0:128, 0:nw],
                                                 krot[h][:, ksl],
                                                 qrot[h][:, qsl],
                                                 start=True, stop=True)
                        for h in range(H):
                            hsl = slice(h * 128, (h + 1) * 128)
                            est = wp.tile([128, c.QC], F16, tag="est", bufs=8,
                                          name="est")
                            nc.scalar.activation(est[0:128, 0:cur_w],
                                                 st_cur[h][0:128, 0:cur_w],
                                                 EXP, scale=scale_s)
                            if cur_j >= 0:
                                nc.vector.tensor_tensor(
                                    est[0:128, 0:128],
                                    est[0:128, 0:128],
                                    masks[:, cur_j, cur_off:cur_off + 128],
                                    MUL)
                            nc.tensor.matmul(
                                ctx_ps[h][0:128, cur_off:c.QC],
                                v_sb[:, kt, hsl], est[0:128, 0:cur_w],
                                start=(kt == 0), stop=(kt == nkt - 1))
                            par = kt & 1
                            ea = eacc[h][par]
                            if kt < 2:
                                nc.vector.tensor_copy(
                                    ea[0:128, cur_off:c.QC],
                                    est[0:128, 0:cur_w])
                            elif h == 0 and ACC_ON_GPSIMD and cur_j < 0:
                                nc.gpsimd.tensor_tensor(
                                    ea[0:128, cur_off:c.QC],
                                    ea[0:128, cur_off:c.QC],
                                    est[0:128, 0:cur_w], ADD)
                            else:
                                nc.vector.tensor_tensor(
                                    ea[0:128, cur_off:c.QC],
                                    ea[0:128, cur_off:c.QC],
                                    est[0:128, 0:cur_w], ADD)
                            weave(WEAVE_RATE)
                    # adapter attention (no rope on adapter k/v) + combine,
                    # stage-interleaved across heads so the serial DVE chain
                    # (recips, t1/t2) overlaps Pool broadcasts and PE weave
                    ast_ps, aest, sums, rcs, bcs, t1s = [], [], [], [], [], []
                    for h in range(H):
                        a = psp.tile([c.L, c.QC], F32, tag="st",
                                     name=f"ast{qc}_{h}")
                        nc.tensor.matmul(a[:], akt[:, h, 0:c.L],
                                         qrot[h][:, sl], start=True, stop=True)
                        ast_ps.append(a)
                        weave(2)
                    for h in range(H):
                        ae = wp.tile([c.L, c.QC], BF16, tag="aest", bufs=4,
                                     name="aest")
                        nc.scalar.activation(ae[:], ast_ps[h][:], EXP,
                                             scale=scale_a)
                        aest.append(ae)
                    for h in range(H):
                        if nkt > 1:  # merge even/odd accumulators off-PE
                            nc.vector.tensor_tensor(eacc[h][0][:],
                                                    eacc[h][0][:],
                                                    eacc[h][1][:], ADD)
                        sum_m = psp.tile([1, c.QC], F32, tag="st",
                                         name=f"summ{qc}_{h}")
                        nc.tensor.matmul(sum_m[:], ones_f16[:],
                                         eacc[h][0][:],
                                         start=True, stop=True)
                        sum_a = psp.tile([1, c.QC], F32, tag="st",
                                         name=f"suma{qc}_{h}")
                        nc.tensor.matmul(sum_a[:], ones_bf[:], aest[h][:],
                                         start=True, stop=True)
                        sums.append((sum_m, sum_a))
                        weave(2)
                    for h in range(H):
                        rc_m = wp.tile([1, c.QC], F32, tag="rc", bufs=4,
                                       name="rc_m")
                        rc_a = wp.tile([1, c.QC], F32, tag="rc", bufs=4,
                                       name="rc_a")
                        nc.vector.reciprocal_approx_fast(rc_m[:], sums[h][0][:])
                        nc.vector.reciprocal_approx_fast(rc_a[:], sums[h][1][:])
                        rcs.append((rc_m, rc_a))
                        weave(1)
                    for h in range(H):
                        rcb = wp.tile([128, c.QC], F32, tag="rcb", bufs=4,
                                      name="rcb")
                        rab = wp.tile([128, c.QC], F32, tag="rcb", bufs=4,
                                      name="rab")
                        nc.gpsimd.partition_broadcast(rcb[:], rcs[h][0][:], 128)
                        nc.gpsimd.partition_broadcast(rab[:], rcs[h][1][:], 128)
                        bcs.append((rcb, rab))
                        weave(1)
                    for h in range(H):
                        t1 = wp.tile([128, c.QC], F32, tag="cmb", bufs=4,
                                     name="c_t1")
                        nc.vector.tensor_tensor(t1[:], ctx_ps[h][:],
                                                bcs[h][0][:], MUL)
                        t1s.append(t1)
                        weave(2)
                    for h in range(H):
                        hsl = slice(h * 128, (h + 1) * 128)
                        actx_ps = psp.tile([128, c.QC], F32, tag="acc",
                                           name=f"actx{qc}_{h}")
                        nc.tensor.matmul(actx_ps[:], av_sb[:, hsl], aest[h][:],
                                         start=True, stop=True)
                        weave(2)
                        t2 = wp.tile([128, c.QC], F32, tag="cmb", bufs=4,
                                     name="c_t2")
                        nc.vector.tensor_tensor(t2[:], actx_ps[:],
                                                bcs[h][1][:], MUL)
                        ct = wp.tile([128, c.QC], F16, tag="ct", bufs=4,
                                     name="ct")
                        nc.vector.tensor_tensor(ct[:], t1s[h][:], t2[:], ADD)
                        ct_tiles[h] = ct

                def make_out_units(qc, ct_tiles):
                    """Output projection for chunk qc as weave units (it only
                    depends on chunk qc's combine, so it can fill the NEXT
                    chunk's attention k-loop)."""
                    sl = slice(qc * c.QC, (qc + 1) * c.QC)
                    units = []
                    state = {}
                    for dp in range(c.nd // 2):
                        for half in range(2):
                            def omm(dp=dp, half=half):
                                if half == 0:
                                    state[dp] = wp.tile(
                                        [128, 2, c.QC], F16, tag="opair",
                                        bufs=3, name=f"op{qc}_{dp}")
                                dt = dp * 2 + half
                                dsl = slice(dt * 128, (dt + 1) * 128)
                                o_ps = psp.tile([128, c.QC], F32, tag="vo",
                                                name=f"o{qc}_{dt}")
                                for h in range(c.hpc):
                                    nc.tensor.matmul(o_ps[:], wot[:, h, dsl],
                                                     ct_tiles[h][:],
                                                     start=(h == 0),
                                                     stop=(h == c.hpc - 1))
                                if dt % 2 == 0:
                                    nc.scalar.copy(state[dp][:, half, :],
                                                   o_ps[:])
                                else:
                                    nc.vector.tensor_copy(state[dp][:, half, :],
                                                          o_ps[:])
                                if half == 1:
                                    nc.sync.dma_start(
                                        out_r[:, 2 * dp:2 * dp + 2, sl],
                                        state[dp][:])
                            units.append(omm)
                    return units

                # ---------- pipeline ----------
                # per chunk: weave {proj(qc+1), out(qc-1)} into attn(qc)'s
                # k-loop; leftovers drain after, keeping PE dense throughout
                WQ.extend(("proj", u) for u in make_qk_units(0))
                WQ.extend(("proj", u) for u in make_v_units(0))
                drain()
                adapter_kv()
                for qc in range(c.ns):
                    if qc + 1 < c.ns:
                        WQ.extend(("proj", u) for u in make_qk_units(qc + 1))
                        WQ.extend(("proj", u) for u in make_v_units(qc + 1))
                    ct_tiles = {}
                    attn_chunk(qc, ct_tiles)
                    WQ.extend(("out", u) for u in make_out_units(qc, ct_tiles))
                    if qc + 1 == c.ns:
                        drain()
                    else:
                        # drain all but a reserve that keeps the next
                        # attention k-loop AND its combine section fed
                        reserve = 2 * (qc + 2) * c.kpq * WEAVE_RATE + 40
                        while len(WQ) > reserve:
                            WQ.popleft()[1]()
                drain()

    nc.compile()
    return nc


def make_in_maps(cfg, hidden_states, Wq, Wk, Wv, Wo, adaption_prompt,
                 adaption_gate, position_ids):
    """Host-side sharding: slice/transpose/cast per core."""
    c = cfg
    x = np.asarray(hidden_states, np.float32)[0]          # [s, d]
    xt = np.ascontiguousarray(x.T).astype(NP_F16)         # [d, s]
    ap = np.asarray(adaption_prompt, np.float32)[0]       # [L, d]
    apt = np.ascontiguousarray(ap.T).astype(NP_F16)       # [d, L]
    gate = float(np.asarray(adaption_gate).reshape(-1)[0])
    gatec = np.full((128, 1), gate, np.float32)
    pos = np.asarray(position_ids).reshape(-1).astype(np.float64)  # [s]
    inv = 1.0 / (cfg.rope_base ** (np.arange(0, c.hd, 2, dtype=np.float64) / c.hd))
    invx = np.concatenate([inv, inv])                     # [hd]
    ang = invx[:, None] * pos[None, :]                    # [hd, s]
    cosd = np.cos(ang).astype(NP_F16)
    # sign of the rotate_half first block folded in: rows 0:64 receive
    # -x2*sin, rows 64:128 receive +x1*sin
    sh = np.sin(inv[:, None] * pos[None, :])              # [hd/2, s]
    sind = np.concatenate([-sh, sh]).astype(NP_F16)
    in_maps = []
    for i in range(c.n_cores):
        rs = slice(i * c.dh, (i + 1) * c.dh)
        in_maps.append({
            "xt": xt,
            "wqt": np.ascontiguousarray(np.asarray(Wq, np.float32)[rs, :].T).astype(NP_F16),
            "wkt": np.ascontiguousarray(np.asarray(Wk, np.float32)[rs, :].T).astype(NP_F16),
            "wvt": np.ascontiguousarray(np.asarray(Wv, np.float32)[rs, :].T).astype(NP_F16),
            "wot": np.ascontiguousarray(np.asarray(Wo, np.float32)[:, rs].T).astype(NP_F16),
            "apt": apt,
            "gatec": gatec,
            "cosd": cosd,
            "sind": sind,
        })
    return in_maps


def assemble_output(cfg, results):
    acc = np.zeros((cfg.d, cfg.s), np.float32)
    for r in results:
        acc += np.asarray(r["out"], np.float32)           # per-core partial [d, s]
    return np.ascontiguousarray(acc.T)[None]              # [1, s, d]


_NC_CACHE = {}


def run(inputs, cfg=None, trace=False):
    cfg = cfg or Cfg()
    key = (cfg.s, cfg.d, cfg.L, cfg.n_cores, cfg.n_heads)
    if key not in _NC_CACHE:
        _NC_CACHE[key] = build(cfg)
    nc = _NC_CACHE[key]
    in_maps = make_in_maps(cfg, **inputs)
    res = run_bass_kernel_spmd(nc, in_maps, core_ids=list(range(cfg.n_cores)),
                               trace=trace)
    out = assemble_output(cfg, res.results)
    return out, res


def kernel(**inputs) -> np.ndarray:
    out, _ = run(inputs)
    return out.astype(np.float32)


# revision 56
# speedup vs baseline: 1.0014x; 1.0014x over previous
"""AdaptedAttention (llama + adaption-prompt) on 8 TRN2 NeuronCores.

Sharding: tensor-parallel over heads (2 heads/core), zero device collectives
(measured AllGather ~92us/rank through this stack, so the cross-core
reduction of the 8 per-core output partials happens on the host).

Per core (all on-device tensors fp16 except PSUM/f32 glue; fp16 beats bf16
on precision at identical PE/DVE throughput, buying error budget):
  - qT/kT/V projections for its 2 heads from a pre-transposed X [d, s]
  - RoPE in [hd, s] layout from HOST-built cos/sin fp16 tables. On-device
    tables would need ACT Sin, and Sin/Exp never share an activation table
    (1283ns reload per switch, every chunk). The rotate-half sign is folded
    into the host sin table; the cross-partition-base products read the
    projection result straight from PSUM (the equal-base-partition verifier
    rule only applies to SBUF+SBUF operand pairs).
  - attention as S^T = K @ Q^T tiles ([k part, q free]); exp on ACT; causal
    handled by skipping k-tiles above the diagonal, streaming only the
    valid q-columns of diagonal tiles (width QC-128j), and a 0/1 mask on
    each diagonal tile's own 128-col triangle
  - NO PE row-sum/broadcast matmuls for softmax denominators: exp tiles
    accumulate into even/odd fp16 tiles (h0 on the idle GPSIMD, h1 on DVE,
    split so neither engine's add rate binds the k-loop), merged by one DVE
    add; one [128,1]-ones matmul -> denominator row; reciprocal_approx_fast
    + gpsimd partition_broadcast replace ones-outer-product broadcasts
  - adapter path (L=10): adaption gate folded into adapter-V at load time
  - output projection -> full-shape partial [d, s] fp16 (half the f32
    store traffic), staged PSUM->SBUF in dt-pairs alternating ACT/DVE,
    written on the SP HWDGE queue (inputs share it; ACT's queue carries
    only small weight/table loads so its sequencer stays on exp)

Scheduling (the core of the speedup): the exp's ~612ns/tile ACT cost vs
852ns of PE work per k-iter pair means a naive k-loop is ACT-round-trip
bound, and every PE gap also resets the tensor engine's p-state ramp
(full 2.4GHz only after 3us of continuous execution). So:
  - both heads' k-loops are interleaved at iteration level (doubles PE work
    per st-slot round trip; the 2-slot PSUM ping-pong never binds)
  - the NEXT chunk's projection matmuls and the PREVIOUS chunk's output-
    projection matmuls are kept in a queue of single-instruction units and
    woven into the k-loop / combine sections one per iteration, keeping the
    PE stream dense end to end (emission-order = dependency-order, so
    deferred producer units are force-emitted just before the diagonal
    section that consumes them)
  - the softmax combine is stage-interleaved across heads so its serial
    DVE chain (recips, t1/t2) overlaps Pool broadcasts and woven PE work
PSUM = exactly 8 banks via 4 rotating tags (qk/st/acc/vo x2).

TimelineSim (HW-calibrated cost model): 185us single-shot, 151us/rep
steady-state (PE ~99% occupied at the fp16 column floor) vs 362us for the
previous kernel. Host-measured wall marginals through the axon tunnel are
noise-dominated (dispatch variance >> kernel time) and not trustworthy.
"""

import math
from collections import deque
import numpy as np

import concourse.bass as bass
import concourse.bacc as bacc
import concourse.mybir as mybir
import concourse.tile as tile
from concourse.bass_utils import run_bass_kernel_spmd

F16 = mybir.dt.float16
BF16 = mybir.dt.bfloat16
F32 = mybir.dt.float32
NP_F16 = np.float16
NP_BF16 = mybir.dt.np(BF16)


class Cfg:
    def __init__(self, s=2048, d=2048, L=10, n_cores=8, n_heads=16, rope_base=10000.0):
        self.s, self.d, self.L = s, d, L
        self.n_cores = n_cores
        self.n_heads = n_heads
        self.rope_base = rope_base
        self.hd = 128                      # head dim (fixed)
        self.hpc = n_heads // n_cores      # heads per core
        self.dh = self.hpc * self.hd       # local head-dim cols per core
        self.nd = d // 128                 # contraction chunks
        self.QC = 512                      # q-chunk width
        self.ns = s // self.QC             # q-chunks
        self.nst = s // 128                # s tiles (k tiles)
        self.kpq = self.QC // 128          # k-tiles per q-chunk
        assert self.hpc * n_cores == n_heads and d % 128 == 0 and s % self.QC == 0


WEAVE_RATE = 1         # proj matmuls woven per attention k-iter
ACC_ON_GPSIMD = True  # est-sum accumulation engine for non-diag tiles


def build(cfg: Cfg, nrep: int = 1):
    c = cfg
    nc = bacc.Bacc(None, target_bir_lowering=False, num_devices=c.n_cores)

    # ---------------- external I/O (per-core shards) ----------------
    xt_d = nc.dram_tensor("xt", [c.d, c.s], F16, kind="ExternalInput")
    wqt_d = nc.dram_tensor("wqt", [c.d, c.dh], F16, kind="ExternalInput")
    wkt_d = nc.dram_tensor("wkt", [c.d, c.dh], F16, kind="ExternalInput")
    wvt_d = nc.dram_tensor("wvt", [c.d, c.dh], F16, kind="ExternalInput")
    wot_d = nc.dram_tensor("wot", [c.dh, c.d], F16, kind="ExternalInput")
    apt_d = nc.dram_tensor("apt", [c.d, c.L], F16, kind="ExternalInput")
    gatec_d = nc.dram_tensor("gatec", [128, 1], F32, kind="ExternalInput")
    cosd_d = nc.dram_tensor("cosd", [c.hd, c.s], F16, kind="ExternalInput")
    sind_d = nc.dram_tensor("sind", [c.hd, c.s], F16, kind="ExternalInput")
    out_d = nc.dram_tensor("out", [c.d, c.s], F16, kind="ExternalOutput")

    # diagonal-tile causal masks: mask[k, j, q] = 1 if k <= q - 128*j
    kk = np.arange(128)[:, None, None]
    jj = np.arange(c.kpq)[None, :, None]
    qq = np.arange(c.QC)[None, None, :]
    masks_np = (kk <= qq - 128 * jj).astype(NP_F16)  # [128, kpq, QC]
    masks_d = nc.inline_tensor(masks_np, name="masks")

    scale_s = 1.0 / math.sqrt(c.hd)        # main attention scale
    scale_a = 1.0 / math.sqrt(c.n_heads)   # adapter scale (faithful to ref)

    EXP = mybir.ActivationFunctionType.Exp
    ADD = mybir.AluOpType.add
    SUB = mybir.AluOpType.subtract
    MUL = mybir.AluOpType.mult

    xt_r = xt_d.rearrange("(t p) s -> p t s", p=128)
    wqt_r = wqt_d.rearrange("(t p) m -> p t m", p=128)
    wkt_r = wkt_d.rearrange("(t p) m -> p t m", p=128)
    wvt_r = wvt_d.rearrange("(t p) m -> p t m", p=128)
    wot_r = wot_d.rearrange("(h p) m -> p h m", p=128)
    apt_r = apt_d.rearrange("(t p) m -> p t m", p=128)
    out_r = out_d.rearrange("(t p) s -> p t s", p=128)

    with tile.TileContext(nc) as tc:
        with (
            tc.tile_pool(name="persist", bufs=1) as pp,
            tc.tile_pool(name="work", bufs=2) as wp,
            tc.tile_pool(name="psum", bufs=2, space="PSUM") as psp,
        ):
            for _rep in range(nrep):
                # ---------- loads ----------
                # weights/tables on the ACT HWDGE queue, xt/wvt/wot on the SP
                # queue; first pieces small so proj(0) starts within ~2us
                wqt = pp.tile([128, c.nd, c.dh], F16, tag="wqt")
                wkt = pp.tile([128, c.nd, c.dh], F16, tag="wkt")
                qtr = max(c.nd // 4, 1)
                xtc = []
                for qc in range(c.ns):
                    xtq = pp.tile([128, c.nd, c.QC], F16, tag=f"xt{qc}",
                                  name=f"xtc{qc}")
                    xtc.append(xtq)
                sl0 = slice(0, c.QC)
                for p in range(0, c.nd, qtr):
                    pe = min(p + qtr, c.nd)
                    nc.scalar.dma_start(wqt[:, p:pe, :], wqt_r[:, p:pe, :])
                    nc.sync.dma_start(xtc[0][:, p:pe, :], xt_r[:, p:pe, sl0])
                    nc.scalar.dma_start(wkt[:, p:pe, :], wkt_r[:, p:pe, :])
                cos_t = pp.tile([c.hd, c.s], F16, tag="cos")
                sin_t = pp.tile([c.hd, c.s], F16, tag="sin")
                nc.scalar.dma_start(cos_t[:], cosd_d[:])
                nc.scalar.dma_start(sin_t[:], sind_d[:])
                wvt = pp.tile([128, c.nd, c.dh], F16, tag="wvt")
                nc.sync.dma_start(wvt[:], wvt_r[:])
                masks = pp.tile([128, c.kpq, c.QC], F16, tag="masks")
                nc.scalar.dma_start(masks[:], masks_d[:])
                gatec = pp.tile([128, 1], F32, tag="gatec")
                nc.scalar.dma_start(gatec[:], gatec_d[:])
                apt = pp.tile([128, c.nd, c.L], F16, tag="apt")
                nc.scalar.dma_start(apt[:], apt_r[:])
                for qc in range(1, c.ns):
                    sl = slice(qc * c.QC, (qc + 1) * c.QC)
                    nc.sync.dma_start(xtc[qc][:], xt_r[:, :, sl])
                wot = pp.tile([128, c.hpc, c.d], F16, tag="wot")
                nc.sync.dma_start(wot[:], wot_r[:])

                ones_f16 = pp.tile([128, 1], F16, tag="ones_f16")
                nc.gpsimd.memset(ones_f16[:], 1.0)
                ones_bf = pp.tile([c.L, 1], BF16, tag="ones_bf")
                nc.gpsimd.memset(ones_bf[:], 1.0)

                # ---------- persistent intermediates ----------
                qrot = [pp.tile([128, c.s], F16, tag=f"qrot{h}", name=f"qrot{h}")
                        for h in range(c.hpc)]
                krot = [pp.tile([128, c.s], F16, tag=f"krot{h}", name=f"krot{h}")
                        for h in range(c.hpc)]
                v_sb = pp.tile([128, c.nst, c.dh], F16, tag="v")
                akt = pp.tile([128, c.hpc, c.L], F16, tag="akt")
                av_sb = pp.tile([c.L, c.dh], BF16, tag="av")

                def rope_emit(dst_tile, sl, ps):
                    # dst = x*cos + rotate_half(x)*sin, with the sign of the
                    # first sin half pre-folded into the host sin table. The
                    # cross-partition-base reads are legal because ps is in
                    # PSUM (the equal-base rule applies to SBUF+SBUF pairs).
                    t1 = wp.tile([128, c.QC], F16, tag="rt", bufs=4, name="r_t1")
                    t2 = wp.tile([128, c.QC], F16, tag="rt", bufs=4, name="r_t2")
                    nc.vector.tensor_tensor(t1[:], ps[:], cos_t[:, sl], MUL)
                    nc.vector.tensor_tensor(t2[0:64], ps[64:128],
                                            sin_t[0:64, sl], MUL)
                    nc.vector.tensor_tensor(t2[64:128], ps[0:64],
                                            sin_t[64:128, sl], MUL)
                    nc.vector.tensor_tensor(dst_tile[:, sl], t1[:], t2[:], ADD)

                def make_qk_units(qc, which=("q", "k")):
                    """Projection matmuls + rope for chunk qc, as single-
                    matmul emission units for weaving into attention. All q
                    units are emitted before all k units so the last chunk's
                    k can be deferred into its own attention k-loop."""
                    units = []
                    sl = slice(qc * c.QC, (qc + 1) * c.QC)
                    state = {}
                    for w in which:
                        wt = wqt if w == "q" else wkt
                        rot = qrot if w == "q" else krot
                        for h in range(c.hpc):
                            hsl = slice(h * 128, (h + 1) * 128)
                            for t in range(c.nd):
                                def mm(h=h, w=w, wt=wt, t=t, hsl=hsl):
                                    if t == 0:
                                        state[(h, w)] = psp.tile(
                                            [128, c.QC], F32, tag="qk",
                                            name=f"p{qc}_{h}{w}")
                                    nc.tensor.matmul(
                                        state[(h, w)][:], wt[:, t, hsl],
                                        xtc[qc][:, t, :],
                                        start=(t == 0), stop=(t == c.nd - 1))
                                units.append(mm)

                            def fin(h=h, w=w, rot=rot):
                                rope_emit(rot[h], sl, state[(h, w)])
                            units.append(fin)
                    return units

                def make_v_units(qc):
                    units = []
                    state = {}
                    for st_i in range(c.kpq):
                        for t in range(c.nd):
                            def vmm(st_i=st_i, t=t):
                                if t == 0:
                                    state[st_i] = psp.tile(
                                        [128, c.dh], F32, tag="vo",
                                        name=f"v{qc}_{st_i}")
                                ssl = slice(st_i * 128, (st_i + 1) * 128)
                                nc.tensor.matmul(
                                    state[st_i][:], xtc[qc][:, t, ssl],
                                    wvt[:, t, :],
                                    start=(t == 0), stop=(t == c.nd - 1))
                            units.append(vmm)

                        def vfin(st_i=st_i):
                            gst = qc * c.kpq + st_i
                            nc.scalar.copy(v_sb[:, gst, :], state[st_i][:])
                        units.append(vfin)
                    return units

                WQ = deque()  # entries: (tag, emit_fn)

                def weave(n):
                    for _ in range(n):
                        if WQ:
                            WQ.popleft()[1]()

                def drain():
                    while WQ:
                        WQ.popleft()[1]()

                def ensure_proj():
                    # emit queued units until no projection units remain;
                    # Tile deps are program-order, so every proj write must
                    # be emitted before attention instructions that read it
                    while any(tag == "proj" for tag, _ in WQ):
                        WQ.popleft()[1]()

                def adapter_kv():
                    for h in range(c.hpc):
                        hsl = slice(h * 128, (h + 1) * 128)
                        a_ps = psp.tile([128, c.L], F32, tag="st",
                                        name=f"akt_ps{h}")
                        for t in range(c.nd):
                            nc.tensor.matmul(a_ps[:], wkt[:, t, hsl],
                                             apt[:, t, :],
                                             start=(t == 0), stop=(t == c.nd - 1))
                        nc.scalar.copy(akt[:, h, 0:c.L], a_ps[:])
                    av_ps = psp.tile([c.L, c.dh], F32, tag="st", name="av_ps")
                    for t in range(c.nd):
                        nc.tensor.matmul(av_ps[:], apt[:, t, :], wvt[:, t, :],
                                         start=(t == 0), stop=(t == c.nd - 1))
                    # fold the adaption gate into adapter-V (per-partition scale)
                    nc.scalar.activation(av_sb[:], av_ps[:],
                                         mybir.ActivationFunctionType.Copy,
                                         scale=gatec[0:c.L, :])

                def attn_chunk(qc, ct_tiles):
                    # both heads' k-loops interleaved at iteration level:
                    # doubles the PE work per st-slot round trip (exp latency
                    # no longer binds) and spreads est accumulation across
                    # Pool (h0) and DVE (h1)
                    sl = slice(qc * c.QC, (qc + 1) * c.QC)
                    nkt = (qc + 1) * c.kpq
                    H = c.hpc
                    ctx_ps = [psp.tile([128, c.QC], F32, tag="acc",
                                       name=f"ctx{qc}_{h}") for h in range(H)]
                    eacc = [[wp.tile([128, c.QC], F16, tag="eacc", bufs=8,
                                     name=f"eacc{qc}_{h}_{par}")
                             for par in range(2)] for h in range(H)]
                    st_ps = [psp.tile([128, c.QC], F32, tag="st",
                                      name=f"st{qc}_{h}_0") for h in range(H)]
                    for h in range(H):
                        nc.tensor.matmul(st_ps[h][:], krot[h][:, 0:128],
                                         qrot[h][:, sl], start=True, stop=True)
                    if qc == 0 and nkt > 1:
                        # chunk 0's kt=1 init write is narrow; zero the odd
                        # accumulators so the skipped columns are defined
                        for h in range(H):
                            nc.gpsimd.memset(eacc[h][1][:], 0.0)

                    def tile_geom(kt):
                        # diagonal tile j: only q-cols >= 128*j are unmasked;
                        # stream just those (plus its own 128-col triangle
                        # still needs the 0/1 mask)
                        j = kt - qc * c.kpq
                        off = max(j, 0) * 128
                        return j, off, c.QC - off

                    for kt in range(nkt):
                        j, off, w = tile_geom(kt)
                        if j == -1 or (nkt == c.kpq and kt == 0):
                            ensure_proj()  # diag k/V writes must be emitted
                            # before their readers below
                        st_cur = [st_ps[h] for h in range(H)]
                        cur_w = w
                        cur_off = off
                        cur_j = j
                        if kt + 1 < nkt:
                            ksl = slice((kt + 1) * 128, (kt + 2) * 128)
                            _, noff, nw = tile_geom(kt + 1)
                            qsl = slice(qc * c.QC + noff, (qc + 1) * c.QC)
                            for h in range(H):
                                st_ps[h] = psp.tile([128, c.QC], F32,
                                                    tag="st",
                                                    name=f"st{qc}_{h}_{kt+1}")
                                nc.tensor.matmul(st_ps[h][0:128, 0:nw],
                                                 krot[h][:, ksl],
                                                 qrot[h][:, qsl],
                                                 start=True, stop=True)
                        for h in range(H):
                            hsl = slice(h * 128, (h + 1) * 128)
                            est = wp.tile([128, c.QC], F16, tag="est", bufs=8,
                                          name="est")
                            nc.scalar.activation(est[0:128, 0:cur_w],
                                                 st_cur[h][0:128, 0:cur_w],
                                                 EXP, scale=scale_s)
                            if cur_j >= 0:
                                nc.vector.tensor_tensor(
                                    est[0:128, 0:128],
                                    est[0:128, 0:128],
                                    masks[:, cur_j, cur_off:cur_off + 128],
                                    MUL)
                            nc.tensor.matmul(
                                ctx_ps[h][0:128, cur_off:c.QC],
                                v_sb[:, kt, hsl], est[0:128, 0:cur_w],
                                start=(kt == 0), stop=(kt == nkt - 1))
                            par = kt & 1
                            ea = eacc[h][par]
                            if kt < 2:
                                nc.vector.tensor_copy(
                                    ea[0:128, cur_off:c.QC],
                                    est[0:128, 0:cur_w])
                            elif h == 0 and ACC_ON_GPSIMD and cur_j < 0:
                                nc.gpsimd.tensor_tensor(
                                    ea[0:128, cur_off:c.QC],
                                    ea[0:128, cur_off:c.QC],
                                    est[0:128, 0:cur_w], ADD)
                            else:
                                nc.vector.tensor_tensor(
                                    ea[0:128, cur_off:c.QC],
                                    ea[0:128, cur_off:c.QC],
                                    est[0:128, 0:cur_w], ADD)
                            weave(WEAVE_RATE)
                    # adapter attention (no rope on adapter k/v) + combine,
                    # stage-interleaved across heads so the serial DVE chain
                    # (recips, t1/t2) overlaps Pool broadcasts and PE weave
                    ast_ps, aest, sums, rcs, bcs, t1s = [], [], [], [], [], []
                    for h in range(H):
                        a = psp.tile([c.L, c.QC], F32, tag="st",
                                     name=f"ast{qc}_{h}")
                        nc.tensor.matmul(a[:], akt[:, h, 0:c.L],
                                         qrot[h][:, sl], start=True, stop=True)
                        ast_ps.append(a)
                        weave(2)
                    for h in range(H):
                        ae = wp.tile([c.L, c.QC], BF16, tag="aest", bufs=4,
                                     name="aest")
                        nc.scalar.activation(ae[:], ast_ps[h][:], EXP,
                                             scale=scale_a)
                        aest.append(ae)
                    for h in range(H):
                        if nkt > 1:  # merge even/odd accumulators off-PE
                            nc.vector.tensor_tensor(eacc[h][0][:],
                                                    eacc[h][0][:],
                                                    eacc[h][1][:], ADD)
                        sum_m = psp.tile([1, c.QC], F32, tag="st",
                                         name=f"summ{qc}_{h}")
                        nc.tensor.matmul(sum_m[:], ones_f16[:],
                                         eacc[h][0][:],
                                         start=True, stop=True)
                        sum_a = psp.tile([1, c.QC], F32, tag="st",
                                         name=f"suma{qc}_{h}")
                        nc.tensor.matmul(sum_a[:], ones_bf[:], aest[h][:],
                                         start=True, stop=True)
                        sums.append((sum_m, sum_a))
                        weave(2)
                    for h in range(H):
                        rc_m = wp.tile([1, c.QC], F32, tag="rc", bufs=4,
                                       name="rc_m")
                        rc_a = wp.tile([1, c.QC], F32, tag="rc", bufs=4,
                                       name="rc_a")
                        nc.vector.reciprocal_approx_fast(rc_m[:], sums[h][0][:])
                        nc.vector.reciprocal_approx_fast(rc_a[:], sums[h][1][:])
                        rcs.append((rc_m, rc_a))
                        weave(1)
                    for h in range(H):
                        rcb = wp.tile([128, c.QC], F32, tag="rcb", bufs=4,
                                      name="rcb")
                        rab = wp.tile([128, c.QC], F32, tag="rcb", bufs=4,
                                      name="rab")
                        nc.gpsimd.partition_broadcast(rcb[:], rcs[h][0][:], 128)
                        nc.gpsimd.partition_broadcast(rab[:], rcs[h][1][:], 128)
                        bcs.append((rcb, rab))
                        weave(1)
                    for h in range(H):
                        t1 = wp.tile([128, c.QC], F32, tag="cmb", bufs=4,
                                     name="c_t1")
                        nc.vector.tensor_tensor(t1[:], ctx_ps[h][:],
                                                bcs[h][0][:], MUL)
                        t1s.append(t1)
                        weave(2)
                    for h in range(H):
                        hsl = slice(h * 128, (h + 1) * 128)
                        actx_ps = psp.tile([128, c.QC], F32, tag="acc",
                                           name=f"actx{qc}_{h}")
                        nc.tensor.matmul(actx_ps[:], av_sb[:, hsl], aest[h][:],
                                         start=True, stop=True)
                        weave(2)
                        t2 = wp.tile([128, c.QC], F32, tag="cmb", bufs=4,
                                     name="c_t2")
                        nc.vector.tensor_tensor(t2[:], actx_ps[:],
                                                bcs[h][1][:], MUL)
                        ct = wp.tile([128, c.QC], F16, tag="ct", bufs=4,
                                     name="ct")
                        nc.vector.tensor_tensor(ct[:], t1s[h][:], t2[:], ADD)
                        ct_tiles[h] = ct

                def make_out_units(qc, ct_tiles):
                    """Output projection for chunk qc as weave units (it only
                    depends on chunk qc's combine, so it can fill the NEXT
                    chunk's attention k-loop)."""
                    sl = slice(qc * c.QC, (qc + 1) * c.QC)
                    units = []
                    state = {}
                    for dp in range(c.nd // 2):
                        for half in range(2):
                            def omm(dp=dp, half=half):
                                if half == 0:
                                    state[dp] = wp.tile(
                                        [128, 2, c.QC], F16, tag="opair",
                                        bufs=3, name=f"op{qc}_{dp}")
                                dt = dp * 2 + half
                                dsl = slice(dt * 128, (dt + 1) * 128)
                                o_ps = psp.tile([128, c.QC], F32, tag="vo",
                                                name=f"o{qc}_{dt}")
                                for h in range(c.hpc):
                                    nc.tensor.matmul(o_ps[:], wot[:, h, dsl],
                                                     ct_tiles[h][:],
                                                     start=(h == 0),
                                                     stop=(h == c.hpc - 1))
                                if dt % 2 == 0:
                                    nc.scalar.copy(state[dp][:, half, :],
                                                   o_ps[:])
                                else:
                                    nc.vector.tensor_copy(state[dp][:, half, :],
                                                          o_ps[:])
                                # last chunk's final pairs: fire per-tile DMAs
                                # so the drain tail shortens
                                if qc == c.ns - 1 and dp >= c.nd // 2 - 2:
                                    nc.sync.dma_start(
                                        out_r[:, dt:dt + 1, sl],
                                        state[dp][:, half:half + 1, :])
                                elif half == 1:
                                    nc.sync.dma_start(
                                        out_r[:, 2 * dp:2 * dp + 2, sl],
                                        state[dp][:])
                            units.append(omm)
                    return units

                # ---------- pipeline ----------
                # per chunk: weave {proj(qc+1), out(qc-1)} into attn(qc)'s
                # k-loop; leftovers drain after, keeping PE dense throughout
                WQ.extend(("proj", u) for u in make_qk_units(0))
                WQ.extend(("proj", u) for u in make_v_units(0))
                drain()
                adapter_kv()
                for qc in range(c.ns):
                    if qc + 1 < c.ns:
                        WQ.extend(("proj", u) for u in make_qk_units(qc + 1))
                        WQ.extend(("proj", u) for u in make_v_units(qc + 1))
                    ct_tiles = {}
                    attn_chunk(qc, ct_tiles)
                    WQ.extend(("out", u) for u in make_out_units(qc, ct_tiles))
                    if qc + 1 == c.ns:
                        drain()
                    else:
                        # drain all but a reserve that keeps the next
                        # attention k-loop AND its combine section fed
                        reserve = 2 * (qc + 2) * c.kpq * WEAVE_RATE + 40
                        while len(WQ) > reserve:
                            WQ.popleft()[1]()
                drain()

    nc.compile()
    return nc


def make_in_maps(cfg, hidden_states, Wq, Wk, Wv, Wo, adaption_prompt,
                 adaption_gate, position_ids):
    """Host-side sharding: slice/transpose/cast per core."""
    c = cfg
    x = np.asarray(hidden_states, np.float32)[0]          # [s, d]
    xt = np.ascontiguousarray(x.T).astype(NP_F16)         # [d, s]
    ap = np.asarray(adaption_prompt, np.float32)[0]       # [L, d]
    apt = np.ascontiguousarray(ap.T).astype(NP_F16)       # [d, L]
    gate = float(np.asarray(adaption_gate).reshape(-1)[0])
    gatec = np.full((128, 1), gate, np.float32)
    pos = np.asarray(position_ids).reshape(-1).astype(np.float64)  # [s]
    inv = 1.0 / (cfg.rope_base ** (np.arange(0, c.hd, 2, dtype=np.float64) / c.hd))
    invx = np.concatenate([inv, inv])                     # [hd]
    ang = invx[:, None] * pos[None, :]                    # [hd, s]
    cosd = np.cos(ang).astype(NP_F16)
    # sign of the rotate_half first block folded in: rows 0:64 receive
    # -x2*sin, rows 64:128 receive +x1*sin
    sh = np.sin(inv[:, None] * pos[None, :])              # [hd/2, s]
    sind = np.concatenate([-sh, sh]).astype(NP_F16)
    in_maps = []
    for i in range(c.n_cores):
        rs = slice(i * c.dh, (i + 1) * c.dh)
        in_maps.append({
            "xt": xt,
            "wqt": np.ascontiguousarray(np.asarray(Wq, np.float32)[rs, :].T).astype(NP_F16),
            "wkt": np.ascontiguousarray(np.asarray(Wk, np.float32)[rs, :].T).astype(NP_F16),
            "wvt": np.ascontiguousarray(np.asarray(Wv, np.float32)[rs, :].T).astype(NP_F16),
            "wot": np.ascontiguousarray(np.asarray(Wo, np.float32)[:, rs].T).astype(NP_F16),
            "apt": apt,
            "gatec": gatec,
            "cosd": cosd,
            "sind": sind,
        })
    return in_maps


def assemble_output(cfg, results):
    acc = np.zeros((cfg.d, cfg.s), np.float32)
    for r in results:
        acc += np.asarray(r["out"], np.float32)           # per-core partial [d, s]
    return np.ascontiguousarray(acc.T)[None]              # [1, s, d]


_NC_CACHE = {}


def run(inputs, cfg=None, trace=False):
    cfg = cfg or Cfg()
    key = (cfg.s, cfg.d, cfg.L, cfg.n_cores, cfg.n_heads)
    if key not in _NC_CACHE:
        _NC_CACHE[key] = build(cfg)
    nc = _NC_CACHE[key]
    in_maps = make_in_maps(cfg, **inputs)
    res = run_bass_kernel_spmd(nc, in_maps, core_ids=list(range(cfg.n_cores)),
                               trace=trace)
    out = assemble_output(cfg, res.results)
    return out, res


def kernel(**inputs) -> np.ndarray:
    out, _ = run(inputs)
    return out.astype(np.float32)
